# revision 22
# baseline (speedup 1.0000x reference)
"""Causal multi-head attention (b=2, n=2048, dim=1024, 16 heads) on 8 trn2
NeuronCores.

Sharding: core j = 4*g + r owns batch g and heads 4r..4r+3 (tensor parallel
over heads within each batch's 4-core group). Each core:
  P1  projects q/k (transposed layout [head_dim, tokens]) and v (natural
      [tokens, head_dim], ones-augmented) for its 4 heads from x^T.
  P2  causal attention per head pair in S^T orientation: S^T = K^T.T-style
      matmul with d=64 contraction row-packed 2 heads/matmul, exp without
      max subtraction (scores are O(1) here), triangular mask on diagonal
      tiles, O'^T = V_aug.T @ expS^T accumulated in PSUM (row 64 = softmax
      denominator Z), normalization by 1/Z broadcast.
  A2A transposes the sharding of A^T = [head_dim*heads, tokens] from
      head-sharded to token-sharded (8-core AllToAll; each core addresses
      its group's chunks via partition_id-derived offsets).
  P3  out = A^T.T @ Wout for this core's 512-token block, plus biases.
Host: transposes x per batch, slices weights per head group, gathers the 8
[512, 1024] row blocks into the full [2, 2048, 1024] output.
"""
import numpy as np

import concourse.bass as bass
import concourse.mybir as mybir
import concourse.tile as tile
from concourse.bass import AP, ds
from concourse.bass_utils import run_bass_kernel_spmd
from concourse.vector_clock import ScopedClock

F32 = mybir.dt.float32
F32R = mybir.dt.float32r
EXP = mybir.ActivationFunctionType.Exp

N_CORES = 8
B, N, DIM, H = 2, 2048, 1024, 16
D = DIM // H                 # 64
HL = 4                       # heads per core
KT = DIM // 128              # 8 contraction k-tiles
NJ = N // 128                # 16 key tiles per batch
NI = N // 512                # 4 query i-blocks per batch
SCALE = float(D) ** -0.5

# float32r: single-pass fp32 matmul (~4x faster at N>=256, slightly reduced
# multiply precision). Set False for full-precision two-pass fp32.
MM_R = False


def _split_multi_waits(nc):
    """This walrus build rejects instructions carrying more than one sync
    wait. Hoist extra waits onto same-engine NoOps inserted directly before
    the offending instruction (engines execute their stream in order, so
    this preserves semantics)."""
    n = 0
    for f in nc.m.functions:
        for bb in f.blocks:
            insts = bb.instructions
            out = []
            changed = False
            for inst in insts:
                si = inst.sync_info
                waits = list(si.on_wait) if si is not None and si.on_wait else []
                if len(waits) > 1:
                    changed = True
                    for w in waits[:-1]:
                        nop = mybir.InstNoOp(name=f"I-waitfix-{n}", ins=[],
                                             outs=[])
                        n += 1
                        nop.engine = inst.engine
                        nop.sync_info = mybir.SyncInfo(on_wait=[w],
                                                       on_update=[])
                        out.append(nop)
                    si.on_wait = waits[-1:]
                out.append(inst)
            if changed:
                insts[:] = out
    return n


class _TC(tile.TileContext):
    """Tail drain in this walrus build only supports one sync-wait per CTRL
    instruction; spread the residual global-clock waits over SP nops, and
    split any remaining multi-wait instructions after scheduling."""

    def _drain_and_barrier(self, tick_clock, wait_clock):
        nop = self.nc.sync.nop()
        wait_clock.add_sem_waits(nop.ins, ScopedClock({None: tick_clock.global_clock}))
        si = nop.ins.sync_info
        waits = list(si.on_wait or []) if si is not None else []
        if len(waits) > 1:
            si.on_wait = waits[:1]
            for w in waits[1:]:
                extra = self.nc.sync.nop()
                extra.ins.sync_info = mybir.SyncInfo(on_wait=[w], on_update=[])
        self.nc.sync.drain()
        self.nc.all_engine_barrier()
        assert self.sems is not None
        popped = self.nc._tile_sem_poison_stack.pop()
        assert popped is self._sem_poison
        self.nc.clear_and_free_semaphores(list(self.sems.allocated().values()))
        self.nc.all_engine_barrier()

    def __exit__(self, exc_type, exc_val, exc_tb):
        r = super().__exit__(exc_type, exc_val, exc_tb)
        if exc_type is None:
            _split_multi_waits(self.nc)
        return r


def _r(ap):
    return ap.bitcast(F32R) if MM_R else ap


def _bcast(src_dram_row, parts):
    """DRAM [1, n] row -> AP replicating it over `parts` partitions (step-0
    leading dim; only legal for DRAM sources)."""
    return AP(src_dram_row.tensor, src_dram_row.offset,
              [[0, parts]] + list(src_dram_row.ap)[1:])


def _build():
    nc = bass.Bass(trn_type="TRN2", target_bir_lowering=False, debug=False,
                   num_devices=N_CORES)
    dt = F32
    xt_d = nc.dram_tensor("xt", [DIM, N], dt, kind="ExternalInput").ap()
    wq_d = nc.dram_tensor("wq", [DIM, HL * D], dt, kind="ExternalInput").ap()
    wk_d = nc.dram_tensor("wk", [DIM, HL * D], dt, kind="ExternalInput").ap()
    wv_d = nc.dram_tensor("wv", [DIM, HL * D], dt, kind="ExternalInput").ap()
    wout_d = nc.dram_tensor("wout", [DIM, DIM], dt, kind="ExternalInput").ap()
    bq_d = nc.dram_tensor("bq", [HL * D, 1], dt, kind="ExternalInput").ap()
    bk_d = nc.dram_tensor("bk", [HL * D, 1], dt, kind="ExternalInput").ap()
    bv_d = nc.dram_tensor("bv", [1, HL * D], dt, kind="ExternalInput").ap()
    bout_d = nc.dram_tensor("bout", [1, DIM], dt, kind="ExternalInput").ap()
    mask_d = nc.dram_tensor("mask", [128, 128], dt, kind="ExternalInput").ap()
    out_d = nc.dram_tensor("out", [N // HL, DIM], dt, kind="ExternalOutput").ap()

    with _TC(nc) as tc:
        _body(nc, tc, xt_d, wq_d, wk_d, wv_d, wout_d, bq_d, bk_d, bv_d,
              bout_d, mask_d, out_d)
    return nc


def _body(nc, tc, xt_d, wq_d, wk_d, wv_d, wout_d, bq_d, bk_d, bv_d, bout_d,
          mask_d, out_d):
    mm = nc.tensor.matmul
    with tc.tile_pool(name="persist", bufs=1) as pers:
        # Persistent SBUF: q^T/k^T per head pair, v (ones-augmented) per
        # 128-token tile, A^T per head pair, mask, biases.
        qt = [pers.tile([128, N], F32, tag=f"qt{p}", name=f"qt{p}") for p in (0, 1)]
        kt = [pers.tile([128, N], F32, tag=f"kt{p}", name=f"kt{p}") for p in (0, 1)]
        vt = [pers.tile([128, HL * (D + 1)], F32, tag=f"v{t}", name=f"v{t}")
              for t in range(NJ)]
        at = [pers.tile([128, N], F32, tag=f"at{p}", name=f"at{p}") for p in (0, 1)]
        mask_sb = pers.tile([128, 128], F32, tag="mask", name="mask_sb")
        bqc = pers.tile([128, 2], F32, tag="bqc", name="bqc")
        bkc = pers.tile([128, 2], F32, tag="bkc", name="bkc")
        bvb = pers.tile([128, HL * D], F32, tag="bvb", name="bvb")
        boutb = pers.tile([128, DIM], F32, tag="boutb", name="boutb")

        nc.sync.dma_start(mask_sb[:], mask_d[:])
        nc.sync.dma_start(bqc[:], bq_d.rearrange("(m p) o -> p (m o)", p=128))
        nc.sync.dma_start(bkc[:], bk_d.rearrange("(m p) o -> p (m o)", p=128))
        nc.sync.dma_start(bvb[:], _bcast(bv_d[0:1, :], 128))
        nc.sync.dma_start(boutb[:], _bcast(bout_d[0:1, :], 128))

        # ---------------- P1: projections ----------------
        with (tc.tile_pool(name="p1s", bufs=1) as p1s,
              tc.tile_pool(name="p1p", bufs=2, space="PSUM") as p1p):
            xt_sb = p1s.tile([128, KT, N], F32, tag="xt", name="xt_sb")
            nc.sync.dma_start(xt_sb[:], xt_d.rearrange("(k p) n -> p k n", p=128))
            w_sb = {}
            for nm, d_ap in (("wq", wq_d), ("wk", wk_d), ("wv", wv_d)):
                w_sb[nm] = p1s.tile([128, KT, HL * D], F32, tag=nm, name=f"{nm}_sb")
                nc.sync.dma_start(w_sb[nm][:],
                                  d_ap.rearrange("(k p) e -> p k e", p=128))

            for w, bcol, dst in (("wq", bqc, qt), ("wk", bkc, kt)):
                for mt in (0, 1):
                    for nt in range(N // 512):
                        ps = p1p.tile([128, 512], F32, tag="pqk", name="ps_qk")
                        for kk in range(KT):
                            mm(ps[:],
                               _r(w_sb[w][:, kk, 128 * mt:128 * mt + 128]),
                               _r(xt_sb[:, kk, 512 * nt:512 * nt + 512]),
                               start=(kk == 0), stop=(kk == KT - 1))
                        nc.vector.tensor_scalar_add(
                            dst[mt][:, 512 * nt:512 * nt + 512], ps[:],
                            bcol[:, mt:mt + 1])

            for tt in range(NJ):
                ps = p1p.tile([128, HL * D], F32, tag="pv", name="ps_v")
                for kk in range(KT):
                    mm(ps[:],
                       _r(xt_sb[:, kk, 128 * tt:128 * tt + 128]),
                       _r(w_sb["wv"][:, kk, :]),
                       start=(kk == 0), stop=(kk == KT - 1))
                vv = vt[tt].rearrange("p (h x) -> p h x", x=D + 1)
                nc.vector.tensor_add(vv[:, :, 0:D],
                                     ps.rearrange("p (h x) -> p h x", x=D),
                                     bvb.rearrange("p (h x) -> p h x", x=D))
                nc.vector.memset(vv[:, :, D:D + 1], 1.0)

        # wout prefetch (xt freed above; load during P2)
        with tc.tile_pool(name="p3w", bufs=1) as p3w:
            wout_sb = p3w.tile([128, KT, DIM], F32, tag="wout", name="wout_sb")
            nc.sync.dma_start(wout_sb[:],
                              wout_d.rearrange("(k p) c -> p k c", p=128))

            # ---------------- P2: attention ----------------
            with (tc.tile_pool(name="p2s", bufs=3) as p2s,
                  tc.tile_pool(name="p2n", bufs=2) as p2n,
                  tc.tile_pool(name="p2d", bufs=2, space="DRAM") as p2d,
                  tc.tile_pool(name="sp", bufs=2, space="PSUM") as sp,
                  tc.tile_pool(name="op", bufs=2, space="PSUM") as op):
                for pp in (0, 1):
                    for I in range(NI):
                        i0 = 512 * I
                        last = 4 * I + 3
                        poA = op.tile([D + 1, 512], F32, tag="oA", name="poA")
                        poB = op.tile([D + 1, 512], F32, tag="oB", name="poB")
                        for jj in range(4 * I + 4):
                            di = jj - 4 * I
                            f0 = 128 * di if di >= 0 else 0
                            ps = sp.tile([128, 1024], F32, tag="s", name="ps_s")
                            mm(ps[:, f0:512],
                               _r(kt[pp][0:64, 128 * jj:128 * jj + 128]),
                               _r(qt[pp][0:64, i0 + f0:i0 + 512]),
                               start=True, stop=True)
                            mm(ps[:, 512 + f0:1024],
                               _r(kt[pp][64:128, 128 * jj:128 * jj + 128]),
                               _r(qt[pp][64:128, i0 + f0:i0 + 512]),
                               start=True, stop=True)
                            e = p2s.tile([128, 1024], F32, tag="e", name="e_s")
                            ev = e.rearrange("p (h x) -> p h x", x=512)
                            pv2 = ps.rearrange("p (h x) -> p h x", x=512)
                            nc.scalar.activation(ev[:, :, f0:512],
                                                 pv2[:, :, f0:512], EXP,
                                                 scale=SCALE)
                            if di >= 0:
                                nc.vector.tensor_mul(ev[:, 0, f0:f0 + 128],
                                                     ev[:, 0, f0:f0 + 128],
                                                     mask_sb[:])
                                nc.vector.tensor_mul(ev[:, 1, f0:f0 + 128],
                                                     ev[:, 1, f0:f0 + 128],
                                                     mask_sb[:])
                            vv = vt[jj].rearrange("p (h x) -> p h x", x=D + 1)
                            mm(poA[:, f0:512], _r(vv[:, 2 * pp, :]),
                               _r(e[:, f0:512]),
                               start=(jj == 0), stop=(jj == last))
                            mm(poB[:, f0:512], _r(vv[:, 2 * pp + 1, :]),
                               _r(e[:, 512 + f0:1024]),
                               start=(jj == 0), stop=(jj == last))
                        # normalization: Z sits in row 64 of each O' psum.
                        # 1/Z computed on partition 64, bounced via DRAM to
                        # broadcast across partitions (step-0 DMA).
                        zrow = p2n.tile([128, 1024], F32, tag="zrow", name="zrow")
                        nc.vector.reciprocal(zrow[64:65, 0:512], poA[64:65, :])
                        nc.vector.reciprocal(zrow[64:65, 512:1024], poB[64:65, :])
                        zdram = p2d.tile([1, 1024], F32, tag="zdram", name="zdram")
                        nc.sync.dma_start(zdram[0:1, :], zrow[64:65, :])
                        rzb = p2n.tile([64, 1024], F32, tag="rzb", name="rzb")
                        nc.sync.dma_start(rzb[:], _bcast(zdram[0:1, :], 64))
                        nc.vector.tensor_mul(at[pp][0:64, i0:i0 + 512],
                                             poA[0:64, :], rzb[:, 0:512])
                        stB = p2n.tile([64, 512], F32, tag="stB", name="stB")
                        nc.vector.tensor_mul(stB[:], poB[0:64, :],
                                             rzb[:, 512:1024])
                        nc.sync.dma_start(at[pp][64:128, i0:i0 + 512], stB[:])

            # ---------------- A2A + P3: output projection ----------------
            pid = nc.sync.partition_id()
            gsel = nc.sync.snap(pid // 4, min_val=0, max_val=1)
            with (tc.tile_pool(name="dram", bufs=1, space="DRAM") as dram,
                  tc.tile_pool(name="p3s", bufs=2) as p3s,
                  tc.tile_pool(name="p3p", bufs=2, space="PSUM") as p3p):
                a2a_in = dram.tile([2048, 512], F32, name="a2a_in")
                a2a_out = dram.tile([2048, 512], F32, name="a2a_out")
                # chunk t (t = 0..3) of my group occupies rows
                # [1024*g + 256*t, +256): first 128 rows from at[0], next 128
                # from at[1]. One strided DMA per source tile, with the group
                # as a dynamically-indexed size-1 block dim, keeps the
                # dynamic-DMA count low (each burns SP bounds-check regs).
                a2a_in_v = a2a_in.rearrange("(G t q) c -> q G t c", t=4, q=256)
                for p in (0, 1):
                    dst = a2a_in_v[128 * p:128 * p + 128, ds(gsel, 1), :, :]
                    src = at[p].rearrange("p (t c) -> p t c", c=512)
                    nc.sync.dma_start(dst, src)
                nc.gpsimd.collective_compute(
                    "AllToAll", mybir.AluOpType.bypass,
                    replica_groups=[list(range(N_CORES))],
                    ins=[a2a_in.opt()], outs=[a2a_out.opt()])
                atf = p3s.tile([128, KT, 512], F32, tag="atf", name="atf", bufs=1)
                a2a_out_v = a2a_out.rearrange("(G k p) c -> p G k c",
                                              k=KT, p=128)
                nc.sync.dma_start(atf[:], a2a_out_v[:, ds(gsel, 1), :, :])
                for it in range(4):
                    for ct in range(2):
                        ps = p3p.tile([128, 512], F32, tag="po", name="ps_o")
                        for kk in range(KT):
                            mm(ps[:],
                               _r(atf[:, kk, 128 * it:128 * it + 128]),
                               _r(wout_sb[:, kk, 512 * ct:512 * ct + 512]),
                               start=(kk == 0), stop=(kk == KT - 1))
                        osb = p3s.tile([128, 512], F32, tag="osb", name="osb")
                        nc.vector.tensor_add(osb[:], ps[:],
                                             boutb[:, 512 * ct:512 * ct + 512])
                        nc.sync.dma_start(
                            out_d[128 * it:128 * it + 128,
                                  512 * ct:512 * ct + 512], osb[:])


_NC_CACHE = {}

# test-only knobs: set TRACE=True before calling kernel() to profile; the
# BassKernelResults of the last run lands in LAST_RESULT.
TRACE = False
LAST_RESULT = None


def _get_nc():
    if "nc" not in _NC_CACHE:
        _NC_CACHE["nc"] = _build()
    return _NC_CACHE["nc"]


def kernel(x, Wq, bq, Wkv, bkv, Wout, bout):
    x = np.asarray(x, np.float32)
    Wq = np.asarray(Wq, np.float32)
    bq = np.asarray(bq, np.float32)
    Wkv = np.asarray(Wkv, np.float32)
    bkv = np.asarray(bkv, np.float32)
    Wout = np.asarray(Wout, np.float32)
    bout = np.asarray(bout, np.float32)

    mask = np.triu(np.ones((128, 128), np.float32))  # mask[p, c] = c >= p
    xts = [np.ascontiguousarray(x[g].T) for g in range(B)]
    in_maps = []
    for j in range(N_CORES):
        g, r = divmod(j, 4)
        cols = slice(HL * D * r, HL * D * (r + 1))
        in_maps.append({
            "xt": xts[g],
            "wq": np.ascontiguousarray(Wq[:, cols]),
            "wk": np.ascontiguousarray(Wkv[:, 0:DIM][:, cols]),
            "wv": np.ascontiguousarray(Wkv[:, DIM:2 * DIM][:, cols]),
            "wout": Wout,
            "bq": np.ascontiguousarray(bq[cols][:, None]),
            "bk": np.ascontiguousarray(bkv[0:DIM][cols][:, None]),
            "bv": np.ascontiguousarray(bkv[DIM:2 * DIM][cols][None, :]),
            "bout": np.ascontiguousarray(bout[None, :]),
            "mask": mask,
        })
    res = run_bass_kernel_spmd(_get_nc(), in_maps, list(range(N_CORES)),
                               trace=TRACE)
    global LAST_RESULT
    LAST_RESULT = res
    out = np.empty((B, N, DIM), np.float32)
    for j in range(N_CORES):
        g, r = divmod(j, 4)
        out[g, 512 * r:512 * (r + 1)] = res.results[j]["out"]
    return out


# revision 25
# speedup vs baseline: 1.6077x; 1.6077x over previous
"""Causal multi-head attention (b=2, n=2048, dim=1024, 16 heads) on 8 trn2
NeuronCores.

Sharding: core j = 4*g + r owns batch g and heads 4r..4r+3 (tensor parallel
over heads within each batch's 4-core group). Each core:
  P1  projects q/k (transposed layout [head_dim, tokens]) and v (natural
      [tokens, head_dim], ones-augmented) for its 4 heads from x^T.
  P2  causal attention per head pair in S^T orientation: S^T = K^T.T-style
      matmul with d=64 contraction row-packed 2 heads/matmul, exp without
      max subtraction (scores are O(1) here), triangular mask on diagonal
      tiles, O'^T = V_aug.T @ expS^T accumulated in PSUM (row 64 = softmax
      denominator Z), normalization by 1/Z broadcast.
  A2A transposes the sharding of A^T = [head_dim*heads, tokens] from
      head-sharded to token-sharded (8-core AllToAll; each core addresses
      its group's chunks via partition_id-derived offsets).
  P3  out = A^T.T @ Wout for this core's 512-token block, plus biases.
Host: transposes x per batch, slices weights per head group, gathers the 8
[512, 1024] row blocks into the full [2, 2048, 1024] output.
"""
import numpy as np

import concourse.bass as bass
import concourse.mybir as mybir
import concourse.tile as tile
from concourse.bass import AP, ds
from concourse.bass_utils import run_bass_kernel_spmd
from concourse.vector_clock import ScopedClock

F32 = mybir.dt.float32
F32R = mybir.dt.float32r
EXP = mybir.ActivationFunctionType.Exp

N_CORES = 8
B, N, DIM, H = 2, 2048, 1024, 16
D = DIM // H                 # 64
HL = 4                       # heads per core
KT = DIM // 128              # 8 contraction k-tiles
NJ = N // 128                # 16 key tiles per batch
NI = N // 512                # 4 query i-blocks per batch
SCALE = float(D) ** -0.5

# float32r: single-pass fp32 matmul (~4x faster at N>=256, slightly reduced
# multiply precision). Set False for full-precision two-pass fp32.
MM_R = True


def _split_multi_waits(nc):
    """This walrus build rejects instructions carrying more than one sync
    wait. Hoist extra waits onto same-engine NoOps inserted directly before
    the offending instruction (engines execute their stream in order, so
    this preserves semantics)."""
    n = 0
    for f in nc.m.functions:
        for bb in f.blocks:
            insts = bb.instructions
            out = []
            changed = False
            for inst in insts:
                si = inst.sync_info
                waits = list(si.on_wait) if si is not None and si.on_wait else []
                if len(waits) > 1:
                    changed = True
                    for w in waits[:-1]:
                        nop = mybir.InstNoOp(name=f"I-waitfix-{n}", ins=[],
                                             outs=[])
                        n += 1
                        nop.engine = inst.engine
                        nop.sync_info = mybir.SyncInfo(on_wait=[w],
                                                       on_update=[])
                        out.append(nop)
                    si.on_wait = waits[-1:]
                out.append(inst)
            if changed:
                insts[:] = out
    return n


class _TC(tile.TileContext):
    """Tail drain in this walrus build only supports one sync-wait per CTRL
    instruction; spread the residual global-clock waits over SP nops, and
    split any remaining multi-wait instructions after scheduling."""

    def _drain_and_barrier(self, tick_clock, wait_clock):
        nop = self.nc.sync.nop()
        wait_clock.add_sem_waits(nop.ins, ScopedClock({None: tick_clock.global_clock}))
        si = nop.ins.sync_info
        waits = list(si.on_wait or []) if si is not None else []
        if len(waits) > 1:
            si.on_wait = waits[:1]
            for w in waits[1:]:
                extra = self.nc.sync.nop()
                extra.ins.sync_info = mybir.SyncInfo(on_wait=[w], on_update=[])
        self.nc.sync.drain()
        self.nc.all_engine_barrier()
        assert self.sems is not None
        popped = self.nc._tile_sem_poison_stack.pop()
        assert popped is self._sem_poison
        self.nc.clear_and_free_semaphores(list(self.sems.allocated().values()))
        self.nc.all_engine_barrier()

    def __exit__(self, exc_type, exc_val, exc_tb):
        r = super().__exit__(exc_type, exc_val, exc_tb)
        if exc_type is None:
            _split_multi_waits(self.nc)
        return r


# dtype for every tensor on a matmul input path (walrus requires producers
# of f32r-matmul operands to be declared f32r themselves; bits are plain
# fp32 either way)
MD = F32R if MM_R else F32


def _r(ap):
    return ap


def _bcast(src_dram_row, parts):
    """DRAM [1, n] row -> AP replicating it over `parts` partitions (step-0
    leading dim; only legal for DRAM sources)."""
    return AP(src_dram_row.tensor, src_dram_row.offset,
              [[0, parts]] + list(src_dram_row.ap)[1:])


def _build():
    nc = bass.Bass(trn_type="TRN2", target_bir_lowering=False, debug=False,
                   num_devices=N_CORES)
    dt = F32
    xt_d = nc.dram_tensor("xt", [DIM, N], MD, kind="ExternalInput").ap()
    wq_d = nc.dram_tensor("wq", [DIM, HL * D], MD, kind="ExternalInput").ap()
    wk_d = nc.dram_tensor("wk", [DIM, HL * D], MD, kind="ExternalInput").ap()
    wv_d = nc.dram_tensor("wv", [DIM, HL * D], MD, kind="ExternalInput").ap()
    wout_d = nc.dram_tensor("wout", [DIM, DIM], MD, kind="ExternalInput").ap()
    bq_d = nc.dram_tensor("bq", [HL * D, 1], dt, kind="ExternalInput").ap()
    bk_d = nc.dram_tensor("bk", [HL * D, 1], dt, kind="ExternalInput").ap()
    bv_d = nc.dram_tensor("bv", [1, HL * D], dt, kind="ExternalInput").ap()
    bout_d = nc.dram_tensor("bout", [1, DIM], dt, kind="ExternalInput").ap()
    mask_d = nc.dram_tensor("mask", [128, 128], MD, kind="ExternalInput").ap()
    ones_d = nc.dram_tensor("ones", [1, HL], MD, kind="ExternalInput").ap()
    out_d = nc.dram_tensor("out", [N // HL, DIM], dt, kind="ExternalOutput").ap()

    with _TC(nc) as tc, \
            nc.allow_low_precision(reason="f32r matmul operand staging"):
        _body(nc, tc, xt_d, wq_d, wk_d, wv_d, wout_d, bq_d, bk_d, bv_d,
              bout_d, mask_d, ones_d, out_d)
    return nc


def _body(nc, tc, xt_d, wq_d, wk_d, wv_d, wout_d, bq_d, bk_d, bv_d, bout_d,
          mask_d, ones_d, out_d):
    mm = nc.tensor.matmul
    with tc.tile_pool(name="persist", bufs=1) as pers:
        # Persistent SBUF: q^T/k^T per head pair, v (ones-augmented) per
        # 128-token tile, A^T per head pair, mask, biases.
        qt = [pers.tile([128, N], MD, tag=f"qt{p}", name=f"qt{p}") for p in (0, 1)]
        kt = [pers.tile([128, N], MD, tag=f"kt{p}", name=f"kt{p}") for p in (0, 1)]
        vt = [pers.tile([128, HL * (D + 1)], MD, tag=f"v{t}", name=f"v{t}")
              for t in range(NJ)]
        at = [pers.tile([128, N], MD, tag=f"at{p}", name=f"at{p}") for p in (0, 1)]
        mask_sb = pers.tile([128, 128], MD, tag="mask", name="mask_sb")
        bqc = pers.tile([128, 2], F32, tag="bqc", name="bqc")
        bkc = pers.tile([128, 2], F32, tag="bkc", name="bkc")
        bvb = pers.tile([128, HL * D], F32, tag="bvb", name="bvb")
        boutb = pers.tile([128, DIM], F32, tag="boutb", name="boutb")

        nc.sync.dma_start(mask_sb[:], mask_d[:])
        nc.sync.dma_start(bqc[:], bq_d.rearrange("(m p) o -> p (m o)", p=128))
        nc.sync.dma_start(bkc[:], bk_d.rearrange("(m p) o -> p (m o)", p=128))
        nc.sync.dma_start(bvb[:], _bcast(bv_d[0:1, :], 128))
        nc.sync.dma_start(boutb[:], _bcast(bout_d[0:1, :], 128))

        # ---------------- P1: projections ----------------
        with (tc.tile_pool(name="p1s", bufs=1) as p1s,
              tc.tile_pool(name="p1p", bufs=2, space="PSUM") as p1p):
            xt_sb = p1s.tile([128, KT, N], MD, tag="xt", name="xt_sb")
            nc.sync.dma_start(xt_sb[:], xt_d.rearrange("(k p) n -> p k n", p=128))
            w_sb = {}
            for nm, d_ap in (("wq", wq_d), ("wk", wk_d), ("wv", wv_d)):
                w_sb[nm] = p1s.tile([128, KT, HL * D], MD, tag=nm, name=f"{nm}_sb")
                nc.sync.dma_start(w_sb[nm][:],
                                  d_ap.rearrange("(k p) e -> p k e", p=128))

            for w, bcol, dst in (("wq", bqc, qt), ("wk", bkc, kt)):
                for mt in (0, 1):
                    for nt in range(N // 512):
                        ps = p1p.tile([128, 512], F32, tag="pqk", name="ps_qk")
                        for kk in range(KT):
                            mm(ps[:],
                               _r(w_sb[w][:, kk, 128 * mt:128 * mt + 128]),
                               _r(xt_sb[:, kk, 512 * nt:512 * nt + 512]),
                               start=(kk == 0), stop=(kk == KT - 1))
                        nc.vector.tensor_scalar_add(
                            dst[mt][:, 512 * nt:512 * nt + 512], ps[:],
                            bcol[:, mt:mt + 1])

            for tt in range(NJ):
                ps = p1p.tile([128, HL * D], F32, tag="pv", name="ps_v")
                for kk in range(KT):
                    mm(ps[:],
                       _r(xt_sb[:, kk, 128 * tt:128 * tt + 128]),
                       _r(w_sb["wv"][:, kk, :]),
                       start=(kk == 0), stop=(kk == KT - 1))
                vv = vt[tt].rearrange("p (h x) -> p h x", x=D + 1)
                nc.vector.tensor_add(vv[:, :, 0:D],
                                     ps.rearrange("p (h x) -> p h x", x=D),
                                     bvb.rearrange("p (h x) -> p h x", x=D))
                ones_src = AP(ones_d.tensor, ones_d.offset,
                              [[0, 128], [1, HL], [1, 1]])
                nc.sync.dma_start(vv[:, :, D:D + 1], ones_src)

        # wout prefetch (xt freed above; load during P2)
        with tc.tile_pool(name="p3w", bufs=1) as p3w:
            wout_sb = p3w.tile([128, KT, DIM], MD, tag="wout", name="wout_sb")
            nc.sync.dma_start(wout_sb[:],
                              wout_d.rearrange("(k p) c -> p k c", p=128))

            # ---------------- P2: attention ----------------
            with (tc.tile_pool(name="p2s", bufs=3) as p2s,
                  tc.tile_pool(name="p2n", bufs=2) as p2n,
                  tc.tile_pool(name="p2d", bufs=2, space="DRAM") as p2d,
                  tc.tile_pool(name="sp", bufs=2, space="PSUM") as sp,
                  tc.tile_pool(name="op", bufs=2, space="PSUM") as op):
                for pp in (0, 1):
                    for I in range(NI):
                        i0 = 512 * I
                        last = 4 * I + 3
                        poA = op.tile([D + 1, 512], F32, tag="oA", name="poA")
                        poB = op.tile([D + 1, 512], F32, tag="oB", name="poB")
                        for jj in range(4 * I + 4):
                            di = jj - 4 * I
                            f0 = 128 * di if di >= 0 else 0
                            ps = sp.tile([128, 1024], F32, tag="s", name="ps_s")
                            mm(ps[:, f0:512],
                               _r(kt[pp][0:64, 128 * jj:128 * jj + 128]),
                               _r(qt[pp][0:64, i0 + f0:i0 + 512]),
                               start=True, stop=True)
                            mm(ps[:, 512 + f0:1024],
                               _r(kt[pp][64:128, 128 * jj:128 * jj + 128]),
                               _r(qt[pp][64:128, i0 + f0:i0 + 512]),
                               start=True, stop=True)
                            e = p2s.tile([128, 1024], MD, tag="e", name="e_s")
                            ev = e.rearrange("p (h x) -> p h x", x=512)
                            pv2 = ps.rearrange("p (h x) -> p h x", x=512)
                            nc.scalar.activation(ev[:, :, f0:512],
                                                 pv2[:, :, f0:512], EXP,
                                                 scale=SCALE)
                            if di >= 0:
                                nc.vector.tensor_mul(ev[:, 0, f0:f0 + 128],
                                                     ev[:, 0, f0:f0 + 128],
                                                     mask_sb[:])
                                nc.vector.tensor_mul(ev[:, 1, f0:f0 + 128],
                                                     ev[:, 1, f0:f0 + 128],
                                                     mask_sb[:])
                            vv = vt[jj].rearrange("p (h x) -> p h x", x=D + 1)
                            mm(poA[:, f0:512], _r(vv[:, 2 * pp, :]),
                               _r(e[:, f0:512]),
                               start=(jj == 0), stop=(jj == last))
                            mm(poB[:, f0:512], _r(vv[:, 2 * pp + 1, :]),
                               _r(e[:, 512 + f0:1024]),
                               start=(jj == 0), stop=(jj == last))
                        # normalization: Z sits in row 64 of each O' psum.
                        # 1/Z computed on partition 64, bounced via DRAM to
                        # broadcast across partitions (step-0 DMA).
                        zrow = p2n.tile([128, 1024], MD, tag="zrow", name="zrow")
                        nc.vector.reciprocal(zrow[64:65, 0:512], poA[64:65, :])
                        nc.vector.reciprocal(zrow[64:65, 512:1024], poB[64:65, :])
                        zdram = p2d.tile([1, 1024], MD, tag="zdram", name="zdram")
                        nc.sync.dma_start(zdram[0:1, :], zrow[64:65, :])
                        rzb = p2n.tile([64, 1024], MD, tag="rzb", name="rzb")
                        nc.sync.dma_start(rzb[:], _bcast(zdram[0:1, :], 64))
                        nc.vector.tensor_mul(at[pp][0:64, i0:i0 + 512],
                                             poA[0:64, :], rzb[:, 0:512])
                        stB = p2n.tile([64, 512], MD, tag="stB", name="stB")
                        nc.vector.tensor_mul(stB[:], poB[0:64, :],
                                             rzb[:, 512:1024])
                        nc.sync.dma_start(at[pp][64:128, i0:i0 + 512], stB[:])

            # ---------------- A2A + P3: output projection ----------------
            pid = nc.sync.partition_id()
            gsel = nc.sync.snap(pid // 4, min_val=0, max_val=1)
            with (tc.tile_pool(name="dram", bufs=1, space="DRAM") as dram,
                  tc.tile_pool(name="p3s", bufs=2) as p3s,
                  tc.tile_pool(name="p3p", bufs=2, space="PSUM") as p3p):
                a2a_in = dram.tile([2048, 512], MD, name="a2a_in")
                a2a_out = dram.tile([2048, 512], MD, name="a2a_out")
                # chunk t (t = 0..3) of my group occupies rows
                # [1024*g + 256*t, +256): first 128 rows from at[0], next 128
                # from at[1]. One strided DMA per source tile, with the group
                # as a dynamically-indexed size-1 block dim, keeps the
                # dynamic-DMA count low (each burns SP bounds-check regs).
                a2a_in_v = a2a_in.rearrange("(G t q) c -> q G t c", t=4, q=256)
                for p in (0, 1):
                    dst = a2a_in_v[128 * p:128 * p + 128, ds(gsel, 1), :, :]
                    src = at[p].rearrange("p (t c) -> p t c", c=512)
                    nc.sync.dma_start(dst, src)
                nc.gpsimd.collective_compute(
                    "AllToAll", mybir.AluOpType.bypass,
                    replica_groups=[list(range(N_CORES))],
                    ins=[a2a_in.opt()], outs=[a2a_out.opt()])
                atf = p3s.tile([128, KT, 512], MD, tag="atf", name="atf", bufs=1)
                a2a_out_v = a2a_out.rearrange("(G k p) c -> p G k c",
                                              k=KT, p=128)
                nc.sync.dma_start(atf[:], a2a_out_v[:, ds(gsel, 1), :, :])
                for it in range(4):
                    for ct in range(2):
                        ps = p3p.tile([128, 512], F32, tag="po", name="ps_o")
                        for kk in range(KT):
                            mm(ps[:],
                               _r(atf[:, kk, 128 * it:128 * it + 128]),
                               _r(wout_sb[:, kk, 512 * ct:512 * ct + 512]),
                               start=(kk == 0), stop=(kk == KT - 1))
                        osb = p3s.tile([128, 512], F32, tag="osb", name="osb")
                        nc.vector.tensor_add(osb[:], ps[:],
                                             boutb[:, 512 * ct:512 * ct + 512])
                        nc.sync.dma_start(
                            out_d[128 * it:128 * it + 128,
                                  512 * ct:512 * ct + 512], osb[:])


_NC_CACHE = {}

# test-only knobs: set TRACE=True before calling kernel() to profile; the
# BassKernelResults of the last run lands in LAST_RESULT.
TRACE = False
LAST_RESULT = None


def _get_nc():
    if "nc" not in _NC_CACHE:
        _NC_CACHE["nc"] = _build()
    return _NC_CACHE["nc"]


def kernel(x, Wq, bq, Wkv, bkv, Wout, bout):
    x = np.asarray(x, np.float32)
    Wq = np.asarray(Wq, np.float32)
    bq = np.asarray(bq, np.float32)
    Wkv = np.asarray(Wkv, np.float32)
    bkv = np.asarray(bkv, np.float32)
    Wout = np.asarray(Wout, np.float32)
    bout = np.asarray(bout, np.float32)

    mask = np.triu(np.ones((128, 128), np.float32))  # mask[p, c] = c >= p
    xts = [np.ascontiguousarray(x[g].T) for g in range(B)]
    in_maps = []
    for j in range(N_CORES):
        g, r = divmod(j, 4)
        cols = slice(HL * D * r, HL * D * (r + 1))
        in_maps.append({
            "xt": xts[g],
            "wq": np.ascontiguousarray(Wq[:, cols]),
            "wk": np.ascontiguousarray(Wkv[:, 0:DIM][:, cols]),
            "wv": np.ascontiguousarray(Wkv[:, DIM:2 * DIM][:, cols]),
            "wout": Wout,
            "bq": np.ascontiguousarray(bq[cols][:, None]),
            "bk": np.ascontiguousarray(bkv[0:DIM][cols][:, None]),
            "bv": np.ascontiguousarray(bkv[DIM:2 * DIM][cols][None, :]),
            "bout": np.ascontiguousarray(bout[None, :]),
            "mask": mask,
            "ones": np.ones((1, HL), np.float32),
        })
    res = run_bass_kernel_spmd(_get_nc(), in_maps, list(range(N_CORES)),
                               trace=TRACE)
    global LAST_RESULT
    LAST_RESULT = res
    out = np.empty((B, N, DIM), np.float32)
    for j in range(N_CORES):
        g, r = divmod(j, 4)
        out[g, 512 * r:512 * (r + 1)] = res.results[j]["out"]
    return out


# revision 26
# speedup vs baseline: 1.8955x; 1.1790x over previous
"""Causal multi-head attention (b=2, n=2048, dim=1024, 16 heads) on 8 trn2
NeuronCores.

Sharding: core j = 4*g + r owns batch g and heads 4r..4r+3 (tensor parallel
over heads within each batch's 4-core group). Each core:
  P1  projects q/k (transposed layout [head_dim, tokens]) and v (natural
      [tokens, head_dim], ones-augmented) for its 4 heads from x^T.
  P2  causal attention per head pair in S^T orientation: S^T = K^T.T-style
      matmul with d=64 contraction row-packed 2 heads/matmul, exp without
      max subtraction (scores are O(1) here), triangular mask on diagonal
      tiles, O'^T = V_aug.T @ expS^T accumulated in PSUM (row 64 = softmax
      denominator Z), normalization by 1/Z broadcast.
  A2A transposes the sharding of A^T = [head_dim*heads, tokens] from
      head-sharded to token-sharded (8-core AllToAll; each core addresses
      its group's chunks via partition_id-derived offsets).
  P3  out = A^T.T @ Wout for this core's 512-token block, plus biases.
Host: transposes x per batch, slices weights per head group, gathers the 8
[512, 1024] row blocks into the full [2, 2048, 1024] output.
"""
import numpy as np

import concourse.bass as bass
import concourse.mybir as mybir
import concourse.tile as tile
from concourse.bass import AP, ds
from concourse.bass_utils import run_bass_kernel_spmd
from concourse.vector_clock import ScopedClock

F32 = mybir.dt.float32
F32R = mybir.dt.float32r
EXP = mybir.ActivationFunctionType.Exp

N_CORES = 8
B, N, DIM, H = 2, 2048, 1024, 16
D = DIM // H                 # 64
HL = 4                       # heads per core
KT = DIM // 128              # 8 contraction k-tiles
NJ = N // 128                # 16 key tiles per batch
NI = N // 512                # 4 query i-blocks per batch
SCALE = float(D) ** -0.5

# float32r: single-pass fp32 matmul (~4x faster at N>=256, slightly reduced
# multiply precision). Set False for full-precision two-pass fp32.
MM_R = True


def _split_multi_waits(nc):
    """This walrus build rejects instructions carrying more than one sync
    wait. Hoist extra waits onto same-engine NoOps inserted directly before
    the offending instruction (engines execute their stream in order, so
    this preserves semantics)."""
    n = 0
    for f in nc.m.functions:
        for bb in f.blocks:
            insts = bb.instructions
            out = []
            changed = False
            for inst in insts:
                si = inst.sync_info
                waits = list(si.on_wait) if si is not None and si.on_wait else []
                if len(waits) > 1:
                    changed = True
                    for w in waits[:-1]:
                        nop = mybir.InstNoOp(name=f"I-waitfix-{n}", ins=[],
                                             outs=[])
                        n += 1
                        nop.engine = inst.engine
                        nop.sync_info = mybir.SyncInfo(on_wait=[w],
                                                       on_update=[])
                        out.append(nop)
                    si.on_wait = waits[-1:]
                out.append(inst)
            if changed:
                insts[:] = out
    return n


class _TC(tile.TileContext):
    """Tail drain in this walrus build only supports one sync-wait per CTRL
    instruction; spread the residual global-clock waits over SP nops, and
    split any remaining multi-wait instructions after scheduling."""

    def _drain_and_barrier(self, tick_clock, wait_clock):
        nop = self.nc.sync.nop()
        wait_clock.add_sem_waits(nop.ins, ScopedClock({None: tick_clock.global_clock}))
        si = nop.ins.sync_info
        waits = list(si.on_wait or []) if si is not None else []
        if len(waits) > 1:
            si.on_wait = waits[:1]
            for w in waits[1:]:
                extra = self.nc.sync.nop()
                extra.ins.sync_info = mybir.SyncInfo(on_wait=[w], on_update=[])
        self.nc.sync.drain()
        self.nc.all_engine_barrier()
        assert self.sems is not None
        popped = self.nc._tile_sem_poison_stack.pop()
        assert popped is self._sem_poison
        self.nc.clear_and_free_semaphores(list(self.sems.allocated().values()))
        self.nc.all_engine_barrier()

    def __exit__(self, exc_type, exc_val, exc_tb):
        r = super().__exit__(exc_type, exc_val, exc_tb)
        if exc_type is None:
            _split_multi_waits(self.nc)
        return r


# dtype for every tensor on a matmul input path (walrus requires producers
# of f32r-matmul operands to be declared f32r themselves; bits are plain
# fp32 either way)
MD = F32R if MM_R else F32


def _r(ap):
    return ap


def _bcast(src_dram_row, parts):
    """DRAM [1, n] row -> AP replicating it over `parts` partitions (step-0
    leading dim; only legal for DRAM sources)."""
    return AP(src_dram_row.tensor, src_dram_row.offset,
              [[0, parts]] + list(src_dram_row.ap)[1:])


def _build():
    nc = bass.Bass(trn_type="TRN2", target_bir_lowering=False, debug=False,
                   num_devices=N_CORES)
    dt = F32
    # pre-tiled on host: [128, KT*width] rows are fully linear so the bulk
    # DMAs run at line rate instead of 1KB-descriptor rate
    xt_d = nc.dram_tensor("xt", [128, KT * N], MD, kind="ExternalInput").ap()
    wq_d = nc.dram_tensor("wq", [128, KT * HL * D], MD, kind="ExternalInput").ap()
    wk_d = nc.dram_tensor("wk", [128, KT * HL * D], MD, kind="ExternalInput").ap()
    wv_d = nc.dram_tensor("wv", [128, KT * HL * D], MD, kind="ExternalInput").ap()
    wout_d = nc.dram_tensor("wout", [128, KT * DIM], MD, kind="ExternalInput").ap()
    bq_d = nc.dram_tensor("bq", [HL * D, 1], dt, kind="ExternalInput").ap()
    bk_d = nc.dram_tensor("bk", [HL * D, 1], dt, kind="ExternalInput").ap()
    bv_d = nc.dram_tensor("bv", [1, HL * D], dt, kind="ExternalInput").ap()
    bout_d = nc.dram_tensor("bout", [1, DIM], dt, kind="ExternalInput").ap()
    mask_d = nc.dram_tensor("mask", [128, 128], MD, kind="ExternalInput").ap()
    ones_d = nc.dram_tensor("ones", [1, HL], MD, kind="ExternalInput").ap()
    out_d = nc.dram_tensor("out", [N // HL, DIM], dt, kind="ExternalOutput").ap()

    with _TC(nc) as tc, \
            nc.allow_low_precision(reason="f32r matmul operand staging"):
        _body(nc, tc, xt_d, wq_d, wk_d, wv_d, wout_d, bq_d, bk_d, bv_d,
              bout_d, mask_d, ones_d, out_d)
    return nc


def _body(nc, tc, xt_d, wq_d, wk_d, wv_d, wout_d, bq_d, bk_d, bv_d, bout_d,
          mask_d, ones_d, out_d):
    mm = nc.tensor.matmul
    with tc.tile_pool(name="persist", bufs=1) as pers:
        # Persistent SBUF: q^T/k^T per head pair, v (ones-augmented) per
        # 128-token tile, A^T per head pair, mask, biases.
        qt = [pers.tile([128, N], MD, tag=f"qt{p}", name=f"qt{p}") for p in (0, 1)]
        kt = [pers.tile([128, N], MD, tag=f"kt{p}", name=f"kt{p}") for p in (0, 1)]
        vt = [pers.tile([128, HL * (D + 1)], MD, tag=f"v{t}", name=f"v{t}")
              for t in range(NJ)]
        at = [pers.tile([128, N], MD, tag=f"at{p}", name=f"at{p}") for p in (0, 1)]
        mask_sb = pers.tile([128, 128], MD, tag="mask", name="mask_sb")
        bqc = pers.tile([128, 2], F32, tag="bqc", name="bqc")
        bkc = pers.tile([128, 2], F32, tag="bkc", name="bkc")
        bvb = pers.tile([128, HL * D], F32, tag="bvb", name="bvb")
        boutb = pers.tile([128, DIM], F32, tag="boutb", name="boutb")

        nc.scalar.dma_start(mask_sb[:], mask_d[:])
        nc.scalar.dma_start(bqc[:], bq_d.rearrange("(m p) o -> p (m o)", p=128))
        nc.scalar.dma_start(bkc[:], bk_d.rearrange("(m p) o -> p (m o)", p=128))
        nc.scalar.dma_start(bvb[:], _bcast(bv_d[0:1, :], 128))
        nc.scalar.dma_start(boutb[:], _bcast(bout_d[0:1, :], 128))

        # ---------------- P1: projections ----------------
        with (tc.tile_pool(name="p1s", bufs=1) as p1s,
              tc.tile_pool(name="p1p", bufs=2, space="PSUM") as p1p):
            xt_sb = p1s.tile([128, KT, N], MD, tag="xt", name="xt_sb")
            nc.gpsimd.dma_start(xt_sb[:],
                                xt_d.rearrange("p (k n) -> p k n", k=KT))
            w_sb = {}
            for nm, d_ap in (("wq", wq_d), ("wk", wk_d), ("wv", wv_d)):
                w_sb[nm] = p1s.tile([128, KT, HL * D], MD, tag=nm, name=f"{nm}_sb")
                nc.gpsimd.dma_start(w_sb[nm][:],
                                    d_ap.rearrange("p (k e) -> p k e", k=KT))

            for w, bcol, dst in (("wq", bqc, qt), ("wk", bkc, kt)):
                for mt in (0, 1):
                    for nt in range(N // 512):
                        ps = p1p.tile([128, 512], F32, tag="pqk", name="ps_qk")
                        for kk in range(KT):
                            mm(ps[:],
                               _r(w_sb[w][:, kk, 128 * mt:128 * mt + 128]),
                               _r(xt_sb[:, kk, 512 * nt:512 * nt + 512]),
                               start=(kk == 0), stop=(kk == KT - 1))
                        nc.vector.tensor_scalar_add(
                            dst[mt][:, 512 * nt:512 * nt + 512], ps[:],
                            bcol[:, mt:mt + 1])

            for tt in range(NJ):
                ps = p1p.tile([128, HL * D], F32, tag="pv", name="ps_v")
                for kk in range(KT):
                    mm(ps[:],
                       _r(xt_sb[:, kk, 128 * tt:128 * tt + 128]),
                       _r(w_sb["wv"][:, kk, :]),
                       start=(kk == 0), stop=(kk == KT - 1))
                vv = vt[tt].rearrange("p (h x) -> p h x", x=D + 1)
                nc.vector.tensor_add(vv[:, :, 0:D],
                                     ps.rearrange("p (h x) -> p h x", x=D),
                                     bvb.rearrange("p (h x) -> p h x", x=D))
                ones_src = AP(ones_d.tensor, ones_d.offset,
                              [[0, 128], [1, HL], [1, 1]])
                nc.scalar.dma_start(vv[:, :, D:D + 1], ones_src)

        # wout prefetch (xt freed above; load during P2)
        with tc.tile_pool(name="p3w", bufs=1) as p3w:
            wout_sb = p3w.tile([128, KT, DIM], MD, tag="wout", name="wout_sb")
            nc.gpsimd.dma_start(wout_sb[:],
                                wout_d.rearrange("p (k c) -> p k c", k=KT))

            # ---------------- P2: attention ----------------
            with (tc.tile_pool(name="p2s", bufs=3) as p2s,
                  tc.tile_pool(name="p2n", bufs=2) as p2n,
                  tc.tile_pool(name="p2d", bufs=2, space="DRAM") as p2d,
                  tc.tile_pool(name="sp", bufs=2, space="PSUM") as sp,
                  tc.tile_pool(name="op", bufs=2, space="PSUM") as op):
                for pp in (0, 1):
                    for I in range(NI):
                        i0 = 512 * I
                        last = 4 * I + 3
                        poA = op.tile([D + 1, 512], F32, tag="oA", name="poA")
                        poB = op.tile([D + 1, 512], F32, tag="oB", name="poB")
                        for jj in range(4 * I + 4):
                            di = jj - 4 * I
                            f0 = 128 * di if di >= 0 else 0
                            ps = sp.tile([128, 1024], F32, tag="s", name="ps_s")
                            mm(ps[:, f0:512],
                               _r(kt[pp][0:64, 128 * jj:128 * jj + 128]),
                               _r(qt[pp][0:64, i0 + f0:i0 + 512]),
                               start=True, stop=True)
                            mm(ps[:, 512 + f0:1024],
                               _r(kt[pp][64:128, 128 * jj:128 * jj + 128]),
                               _r(qt[pp][64:128, i0 + f0:i0 + 512]),
                               start=True, stop=True)
                            e = p2s.tile([128, 1024], MD, tag="e", name="e_s")
                            ev = e.rearrange("p (h x) -> p h x", x=512)
                            pv2 = ps.rearrange("p (h x) -> p h x", x=512)
                            nc.scalar.activation(ev[:, :, f0:512],
                                                 pv2[:, :, f0:512], EXP,
                                                 scale=SCALE)
                            if di >= 0:
                                nc.vector.tensor_mul(ev[:, 0, f0:f0 + 128],
                                                     ev[:, 0, f0:f0 + 128],
                                                     mask_sb[:])
                                nc.vector.tensor_mul(ev[:, 1, f0:f0 + 128],
                                                     ev[:, 1, f0:f0 + 128],
                                                     mask_sb[:])
                            vv = vt[jj].rearrange("p (h x) -> p h x", x=D + 1)
                            mm(poA[:, f0:512], _r(vv[:, 2 * pp, :]),
                               _r(e[:, f0:512]),
                               start=(jj == 0), stop=(jj == last))
                            mm(poB[:, f0:512], _r(vv[:, 2 * pp + 1, :]),
                               _r(e[:, 512 + f0:1024]),
                               start=(jj == 0), stop=(jj == last))
                        # normalization: Z sits in row 64 of each O' psum.
                        # 1/Z computed on partition 64, bounced via DRAM to
                        # broadcast across partitions (step-0 DMA).
                        zrow = p2n.tile([128, 1024], MD, tag="zrow", name="zrow")
                        nc.vector.reciprocal(zrow[64:65, 0:512], poA[64:65, :])
                        nc.vector.reciprocal(zrow[64:65, 512:1024], poB[64:65, :])
                        zdram = p2d.tile([1, 1024], MD, tag="zdram", name="zdram")
                        nc.sync.dma_start(zdram[0:1, :], zrow[64:65, :])
                        rzb = p2n.tile([64, 1024], MD, tag="rzb", name="rzb")
                        nc.sync.dma_start(rzb[:], _bcast(zdram[0:1, :], 64))
                        nc.vector.tensor_mul(at[pp][0:64, i0:i0 + 512],
                                             poA[0:64, :], rzb[:, 0:512])
                        stB = p2n.tile([64, 512], MD, tag="stB", name="stB")
                        nc.vector.tensor_mul(stB[:], poB[0:64, :],
                                             rzb[:, 512:1024])
                        nc.sync.dma_start(at[pp][64:128, i0:i0 + 512], stB[:])

            # ---------------- A2A + P3: output projection ----------------
            pid = nc.sync.partition_id()
            gsel = nc.sync.snap(pid // 4, min_val=0, max_val=1)
            with (tc.tile_pool(name="dram", bufs=1, space="DRAM") as dram,
                  tc.tile_pool(name="p3s", bufs=2) as p3s,
                  tc.tile_pool(name="p3p", bufs=2, space="PSUM") as p3p):
                a2a_in = dram.tile([2048, 512], MD, name="a2a_in")
                a2a_out = dram.tile([2048, 512], MD, name="a2a_out")
                # chunk t (t = 0..3) of my group occupies rows
                # [1024*g + 256*t, +256): first 128 rows from at[0], next 128
                # from at[1]. One strided DMA per source tile, with the group
                # as a dynamically-indexed size-1 block dim, keeps the
                # dynamic-DMA count low (each burns SP bounds-check regs).
                a2a_in_v = a2a_in.rearrange("(G t q) c -> q G t c", t=4, q=256)
                for p in (0, 1):
                    dst = a2a_in_v[128 * p:128 * p + 128, ds(gsel, 1), :, :]
                    src = at[p].rearrange("p (t c) -> p t c", c=512)
                    nc.sync.dma_start(dst, src)
                nc.gpsimd.collective_compute(
                    "AllToAll", mybir.AluOpType.bypass,
                    replica_groups=[list(range(N_CORES))],
                    ins=[a2a_in.opt()], outs=[a2a_out.opt()])
                atf = p3s.tile([128, KT, 512], MD, tag="atf", name="atf", bufs=1)
                a2a_out_v = a2a_out.rearrange("(G k p) c -> p G k c",
                                              k=KT, p=128)
                nc.sync.dma_start(atf[:], a2a_out_v[:, ds(gsel, 1), :, :])
                for it in range(4):
                    for ct in range(2):
                        ps = p3p.tile([128, 512], F32, tag="po", name="ps_o")
                        for kk in range(KT):
                            mm(ps[:],
                               _r(atf[:, kk, 128 * it:128 * it + 128]),
                               _r(wout_sb[:, kk, 512 * ct:512 * ct + 512]),
                               start=(kk == 0), stop=(kk == KT - 1))
                        osb = p3s.tile([128, 512], F32, tag="osb", name="osb")
                        nc.vector.tensor_add(osb[:], ps[:],
                                             boutb[:, 512 * ct:512 * ct + 512])
                        nc.sync.dma_start(
                            out_d[128 * it:128 * it + 128,
                                  512 * ct:512 * ct + 512], osb[:])


_NC_CACHE = {}

# test-only knobs: set TRACE=True before calling kernel() to profile; the
# BassKernelResults of the last run lands in LAST_RESULT.
TRACE = False
LAST_RESULT = None


def _get_nc():
    if "nc" not in _NC_CACHE:
        _NC_CACHE["nc"] = _build()
    return _NC_CACHE["nc"]


def kernel(x, Wq, bq, Wkv, bkv, Wout, bout):
    x = np.asarray(x, np.float32)
    Wq = np.asarray(Wq, np.float32)
    bq = np.asarray(bq, np.float32)
    Wkv = np.asarray(Wkv, np.float32)
    bkv = np.asarray(bkv, np.float32)
    Wout = np.asarray(Wout, np.float32)
    bout = np.asarray(bout, np.float32)

    def ktile(a):  # [128*KT_rows, width] -> [128, KT_rows*width], row-linear
        kk = a.shape[0] // 128
        return np.ascontiguousarray(
            a.reshape(kk, 128, a.shape[1]).transpose(1, 0, 2).reshape(128, -1))

    mask = np.triu(np.ones((128, 128), np.float32))  # mask[p, c] = c >= p
    xts = [ktile(np.ascontiguousarray(x[g].T)) for g in range(B)]
    wout_t = ktile(Wout)
    in_maps = []
    for j in range(N_CORES):
        g, r = divmod(j, 4)
        cols = slice(HL * D * r, HL * D * (r + 1))
        in_maps.append({
            "xt": xts[g],
            "wq": ktile(Wq[:, cols]),
            "wk": ktile(Wkv[:, 0:DIM][:, cols]),
            "wv": ktile(Wkv[:, DIM:2 * DIM][:, cols]),
            "wout": wout_t,
            "bq": np.ascontiguousarray(bq[cols][:, None]),
            "bk": np.ascontiguousarray(bkv[0:DIM][cols][:, None]),
            "bv": np.ascontiguousarray(bkv[DIM:2 * DIM][cols][None, :]),
            "bout": np.ascontiguousarray(bout[None, :]),
            "mask": mask,
            "ones": np.ones((1, HL), np.float32),
        })
    res = run_bass_kernel_spmd(_get_nc(), in_maps, list(range(N_CORES)),
                               trace=TRACE)
    global LAST_RESULT
    LAST_RESULT = res
    out = np.empty((B, N, DIM), np.float32)
    for j in range(N_CORES):
        g, r = divmod(j, 4)
        out[g, 512 * r:512 * (r + 1)] = res.results[j]["out"]
    return out
